# revision 53
# baseline (speedup 1.0000x reference)
"""Trainium2 Bass kernel for nn_BiLSTM_CRF (CRF negative log-likelihood loss).

Problem: loss = mean_b( logZ_b - gold_b ) for a linear-chain CRF with
B=512 sequences, T=512 steps, K=128 tags (START=126, STOP=127).

Algorithm (per core, data-parallel over batch, 64 sequences/core):

  Z_b = stop^T (prod_{t=T-1..0} D_t W) a0 in the exp domain, where
  W = exp(transitions - c) (c a constant per-step shift keeping magnitudes
  in bf16 range), D_t = diag(exp(feats_t)), a0 = onehot(START).

  The T=512 serial scan is the latency wall (each step is a matmul + an
  elementwise multiply, ~0.5us of fixed pipeline+semaphore latency). We
  break it with a *chunked rank-1 factorization*: products of positive
  matrices contract to rank 1 exponentially fast (top-two Lyapunov
  exponent gap ~0.1/step for these lognormal entries), so splitting T
  into C=32 chunks of L=16,

      P_c ~= (P_c 1)(1^T P_c) / (1^T P_c 1)        (interior seams)

  turns the one 512-step chain into 2C-1=63 *independent* chains of 16
  steps: a forward scan u_c = P_c 1 per chunk (chunk 0 seeded with the
  exact a0) and a backward scan v_c^T = 1^T P_c per chunk c>=1. Measured
  end-to-end seam+quantization error: rel 1.4e-4 on the loss (gate 2e-2).

  Per global step, the 63 chain states are stacked column-wise into 8
  groups of <=512 cols (one PSUM bank each): 8 matmuls (stationary
  exp(T^T-c) fwd / exp(T-c) bwd) + 8 evacuate-multiply ops split across
  the engines that can read PSUM: 4 groups go DVE tensor_tensor
  (psum * expF -> bf16), 4 groups go ACT-copy (psum -> bf16) + Pool-mult
  (bf16 * expF -> bf16), keeping all three elementwise engines busy.

  exp(feats) is precomputed on host, quantized fp8e4m3 (the DVE/Pool ops
  run 1x regardless because of the fp32 PSUM operand, so fp8 costs
  nothing and quarters the DMA: 4MB/core), and shipped ONCE in
  "both-ends-inward" pair order: pair p carries timesteps j=p and
  j=L-1-p of every chunk, which is exactly what fwd(step p)+bwd(step p)
  consume -- the scan is never DMA-starved.

  Gold-path score (transition table gather + emission gather) and the
  final log-dot seam assembly are O(B*T) host-side gathers/reductions
  on the already-present inputs (the device ships back u [128,2048] and
  v [128,1984] per core).
"""

import numpy as np
import ml_dtypes

import concourse.bass as bass
from concourse import bacc
import concourse.mybir as mybir
import concourse.tile as tile
from concourse.tile import add_dep_helper
from concourse.alu_op_type import AluOpType

B, T, K = 512, 512, 128
NCORES = 8
BPC = B // NCORES  # 64 sequences per core
START, STOP = K - 2, K - 1

C_SHIFT = 5.826096

C = 32           # chunks
L = T // C       # 16 scan steps
L2 = L // 2      # 8 DMA pair-blocks
PAIR_COLS = 2 * C * BPC   # cols per DMA pair-block (4096)
S_PROBE = 2      # backward seam-probe length (error ~ (lam2/lam1)^S per seam)

# chunk ranges per group: fwd covers chunks 0..31; probe waves cover 1..31
# groups 0+1 evacuate as one merged DVE tensor_tensor [128,1024] (psum x expf);
# groups 2+3 evacuate via one merged ACT copy psum->bf16, then the multiply
# splits: chunks 16..24 on Pool, 24..28 on Pool, 28..32 on DVE (balance).
FWD_GROUPS = [(0, 8), (8, 16), (16, 24), (24, 32)]
PROBE_WAVES = [(1, 9), (9, 17), (17, 25), (25, 32)]
WARMUP_MMS = 10  # dense PE burst in the DMA lead-in: HAM to K=8/8 (~3.4us)

F32 = mybir.dt.float32
BF16 = mybir.dt.bfloat16
FP8 = mybir.dt.float8e4

_NC_CACHE = {}


def _expf_base(p, slot, c0):
    """Column offset of (pair p, slot, chunk c0) in the expf ship layout
    [K, L2, 2, C, BPC] flattened to [K, L2*2*C*BPC]."""
    return ((p * 2 + slot) * C + c0) * BPC


def _fwd_slice(i, c0, c1):
    """expf cols multiplying the fwd chains (chunks c0:c1) at step i (t=c*L+i)."""
    p, slot = (i, 0) if i < L2 else (L - 1 - i, 1)
    return _expf_base(p, slot, c0), (c1 - c0) * BPC


def _bwd_slice(i, c0, c1):
    """expf cols multiplying the bwd chains at step i (t = c*L + L-1-i)."""
    j = L - 1 - i
    p, slot = (i, 1) if i < L2 else (j, 0)
    return _expf_base(p, slot, c0), (c1 - c0) * BPC


def build_kernel():
    key = "nc"
    if key in _NC_CACHE:
        return _NC_CACHE[key]
    nc = bacc.Bacc(None, target_bir_lowering=False)
    AF = mybir.ActivationFunctionType

    expf_d = nc.dram_tensor("expf", [K, T * BPC], FP8, kind="ExternalInput")
    # host-precomputed stationary weights exp(T^T - c) | exp(T - c), bf16
    wfb_d = nc.dram_tensor("wfb", [K, 2 * K], BF16, kind="ExternalInput")
    seed0_d = nc.dram_tensor("seed0", [K, BPC], BF16, kind="ExternalInput")
    u_d = nc.dram_tensor("u", [K, C * BPC], BF16, kind="ExternalOutput")
    v_d = nc.dram_tensor("v", [K, (C - 1) * BPC], BF16, kind="ExternalOutput")

    with tile.TileContext(nc) as tc:
        with (
            tc.tile_pool(name="const", bufs=1) as cpool,
            tc.tile_pool(name="big", bufs=1) as bigpool,
            tc.tile_pool(name="state", bufs=3) as spool,
            tc.tile_pool(name="raw", bufs=3) as rpool,
            tc.tile_pool(name="psum", bufs=1, space="PSUM") as psum_pool,
        ):
            # ---- constants: stationary weights (host pre-exponentiated) ----
            Wfb = cpool.tile([K, 2 * K], BF16)
            nc.scalar.dma_start(out=Wfb, in_=wfb_d[:])
            Wf = Wfb[:, 0:K]        # exp(T^T - c): lhsT for fwd (out = W @ A)
            Wb = Wfb[:, K : 2 * K]  # exp(T  - c): lhsT for bwd (out = W^T @ m)

            # ---- resident expF, fp8, pair-block order ----
            # trigger order matters: each dma_start eats ~1us of trigger-queue
            # time, so pair 0 (the step-0/probe-seed data) goes absolutely
            # first; pairs 4-7 ride the scalar queue to parallelize triggers
            expf_s = bigpool.tile([K, T * BPC], FP8)
            def pair_dma(eng, p):
                eng.dma_start(
                    out=expf_s[:, p * PAIR_COLS : (p + 1) * PAIR_COLS],
                    in_=expf_d[:, p * PAIR_COLS : (p + 1) * PAIR_COLS],
                )
            q4 = PAIR_COLS // 4
            for s4 in range(4):
                eng = nc.sync if s4 % 2 == 0 else nc.scalar
                eng.dma_start(
                    out=expf_s[:, s4 * q4 : (s4 + 1) * q4],
                    in_=expf_d[:, s4 * q4 : (s4 + 1) * q4],
                )

            # ---- fwd seed: chunk 0 = onehot(START) (shipped), rest = 1.0 ----
            seedF = cpool.tile([K, C * BPC], BF16)
            nc.vector.memset(seedF[:, BPC:], 1.0)
            nc.sync.dma_start(out=seedF[:, 0:BPC], in_=seed0_d[:])
            for p in range(1, 4):
                pair_dma(nc.sync, p)
            for p in range(4, L2):
                pair_dma(nc.scalar, p)

            # ---- per-group psum banks (one each, reused across steps) ----
            # banks 0-3: fwd groups; adjacent pairs are merged-evac'd, so
            # allocate as two [K,1024] tiles (2 banks each) and slice.
            psum_f01 = psum_pool.tile([K, 16 * BPC], F32, name="pf01")
            psum_f23 = psum_pool.tile([K, 16 * BPC], F32, name="pf23")
            psum_f = [
                psum_f01[:, 0 : 8 * BPC],
                psum_f01[:, 8 * BPC : 16 * BPC],
                psum_f23[:, 0 : 8 * BPC],
                psum_f23[:, 8 * BPC : 16 * BPC],
            ]
            # two probe banks, ping-ponged by the 4 probe waves
            psum_p = [
                psum_pool.tile([K, 8 * BPC], F32, name=f"pp{w}") for w in range(2)
            ]

            fwd_state = [
                seedF[:, c0 * BPC : c1 * BPC] for (c0, c1) in FWD_GROUPS
            ]
            ustage = cpool.tile([K, 2 * 16 * BPC], BF16, name="ustage")


            # probe seeds m0 = bf16(expf[j=L-1]) for all 4 waves: DVE casts in
            # the DMA lead-in (only pair 0 needed), so no wave ever stalls
            # the scan TTs behind it in the strict DVE FIFO
            probe_m0 = []
            for wv, (c0, c1) in enumerate(PROBE_WAVES):
                w = (c1 - c0) * BPC
                base, _ = _bwd_slice(0, c0, c1)
                m0 = cpool.tile([K, w], BF16, name=f"pm0_{wv}")
                nc.gpsimd.tensor_copy(m0, expf_s[:, base : base + w])
                probe_m0.append(m0)

            def emit_probe_wave(wv):
                """Backward seam probe v~_c = 1^T(last S_PROBE steps of chunk c)
                for chunks [c0, c1); all ops on DVE + one ACT evac."""
                c0, c1 = PROBE_WAVES[wv]
                w = (c1 - c0) * BPC
                pp = psum_p[wv % 2][:, 0:w]
                m = probe_m0[wv]
                for i in range(S_PROBE):
                    nc.tensor.matmul(pp, Wb, m, start=True, stop=True)
                    if i < S_PROBE - 1:
                        base, _ = _bwd_slice(i + 1, c0, c1)
                        praw = rpool.tile(
                            [K, w], BF16, name=f"pr{wv}_{i}", tag=f"pr{wv % 2}"
                        )
                        nc.scalar.copy(praw, pp)
                        m = spool.tile(
                            [K, w], BF16, name=f"pm{wv}_{i}", tag=f"pb{wv % 2}"
                        )
                        nc.gpsimd.tensor_mul(
                            m, praw, expf_s[:, base : base + w]
                        )
                vt = cpool.tile([K, w], BF16, name=f"v{wv}")
                nc.scalar.copy(vt, pp)
                nc.sync.dma_start(
                    out=v_d[:, (c0 - 1) * BPC : (c1 - 1) * BPC], in_=vt
                )

            # ---- the scan: L global steps, 4 fwd chains + staggered probes ----
            W16 = 16 * BPC
            prev_tt23 = None
            for i in range(L):
                for g, (c0, c1) in enumerate(FWD_GROUPS):
                    nc.tensor.matmul(
                        psum_f[g], Wf, fwd_state[g], start=True, stop=True
                    )
                base01, _ = _fwd_slice(i, 0, 16)
                if i == L - 1:
                    new01 = ustage[:, 0:W16]
                else:
                    new01 = spool.tile([K, W16], BF16, name=f"A01_{i}", tag="af01")
                tt01 = nc.vector.tensor_mul(
                    new01, psum_f01, expf_s[:, base01 : base01 + W16]
                )
                fwd_state[0] = new01[:, 0 : 8 * BPC]
                fwd_state[1] = new01[:, 8 * BPC : W16]

                base23, _ = _fwd_slice(i, 16, 32)
                if i == L - 1:
                    new23 = ustage[:, W16 : 2 * W16]
                else:
                    new23 = spool.tile([K, W16], BF16, name=f"A23_{i}", tag="af23")
                tt23 = nc.vector.tensor_mul(
                    new23, psum_f23, expf_s[:, base23 : base23 + W16]
                )
                fwd_state[2] = new23[:, 0 : 8 * BPC]
                fwd_state[3] = new23[:, 8 * BPC : W16]
                # scheduler alternation edges: neither merged chain may run
                # ahead (a drifting chain ends the kernel latency-bound)
                add_dep_helper(
                    tt23.ins, tt01.ins, sync=False, reason="alternation"
                )
                if prev_tt23 is not None:
                    add_dep_helper(
                        tt01.ins, prev_tt23.ins, sync=False,
                        reason="alternation",
                    )
                prev_tt23 = tt23
                if i < 4:
                    emit_probe_wave(i)

            # ---- ship u: striped transfers over all three trigger queues ----
            NSTR = 9
            step_c = (2 * 16 * BPC) // NSTR
            engs = [nc.sync, nc.scalar, nc.gpsimd]
            pos = 0
            for si in range(NSTR):
                end = 2 * 16 * BPC if si == NSTR - 1 else pos + step_c
                engs[si % 3].dma_start(out=u_d[:, pos:end], in_=ustage[:, pos:end])
                pos = end

    nc.compile()
    nc.finalize()
    _NC_CACHE[key] = nc
    return nc


def prep_inputs(feats, tags, transitions):
    """Host-side marshalling: expF fp8 in pair-block order, shifted transitions."""
    tr = np.asarray(transitions, dtype=np.float32)
    wfb = np.exp(
        np.concatenate([np.ascontiguousarray(tr.T), tr], axis=1)
        - np.float32(C_SHIFT)
    ).astype(ml_dtypes.bfloat16)
    wfb = np.ascontiguousarray(wfb)

    seed0 = np.zeros((K, BPC), dtype=ml_dtypes.bfloat16)
    seed0[START, :] = 1.0
    feats_bf = np.asarray(feats, dtype=np.float32).astype(ml_dtypes.bfloat16)
    in_maps = []
    for core in range(NCORES):
        fc = feats_bf[core * BPC : (core + 1) * BPC]  # [BPC, T, K]
        expF16 = np.exp(fc.astype(np.float32)).astype(ml_dtypes.bfloat16)
        expF = expF16.astype(ml_dtypes.float8_e4m3)
        y = expF.transpose(2, 1, 0).reshape(K, C, L, BPC)  # [K, c, j, b]
        arr = np.empty((K, L2, 2, C, BPC), dtype=ml_dtypes.float8_e4m3)
        arr[:, :, 0] = y[:, :, :L2].transpose(0, 2, 1, 3)          # j = p
        arr[:, :, 1] = y[:, :, L - 1 : L2 - 1 : -1].transpose(0, 2, 1, 3)  # j = L-1-p
        in_maps.append(
            {"expf": np.ascontiguousarray(arr.reshape(K, T * BPC)),
             "wfb": wfb,
             "seed0": seed0}
        )
    return in_maps


def combine_outputs(results, feats, tags, transitions):
    """Host: seam assembly of logZ from u/v + exact gold score; fp64."""
    tr = np.asarray(transitions, dtype=np.float64)
    tags64 = np.asarray(tags).astype(np.int64)
    stopv = np.exp(tr[STOP, :] - C_SHIFT)  # [K]

    # gold score (exact, host)
    ext = np.concatenate(
        [np.full((B, 1), START, np.int64), tags64], axis=1
    )
    trans_gold = tr[ext[:, 1:], ext[:, :-1]].sum(axis=1) + tr[STOP, ext[:, -1]]
    emit_gold = np.take_along_axis(
        np.asarray(feats, dtype=np.float64), tags64[:, :, None], axis=2
    )[..., 0].sum(axis=1)
    gold = trans_gold + emit_gold

    total = 0.0
    for core in range(NCORES):
        u = results[core]["u"].astype(np.float64)   # [K, C*BPC]
        v = results[core]["v"].astype(np.float64)   # [K, (C-1)*BPC]
        u = u.reshape(K, C, BPC)
        v = v.reshape(K, C - 1, BPC)
        logZ = np.log(np.einsum("k,kb->b", stopv, u[:, C - 1]))
        for c in range(1, C):
            logZ += np.log(np.einsum("kb,kb->b", v[:, c - 1], u[:, c - 1]))
            logZ -= np.log(v[:, c - 1].sum(axis=0))
        logZ += (T + 1) * C_SHIFT
        total += float(np.sum(logZ - gold[core * BPC : (core + 1) * BPC]))
    return np.asarray(total / B, dtype=np.float32)


def kernel(feats, tags, transitions):
    from concourse.bass_utils import run_bass_kernel_spmd

    nc = build_kernel()
    in_maps = prep_inputs(feats, tags, transitions)
    res = run_bass_kernel_spmd(nc, in_maps, list(range(NCORES)))
    return combine_outputs(res.results, feats, tags, transitions)


if __name__ == "__main__":
    nc = build_kernel()
    print("kernel built and compiled OK")


# revision 54
# speedup vs baseline: 1.0734x; 1.0734x over previous
"""Trainium2 Bass kernel for nn_BiLSTM_CRF (CRF negative log-likelihood loss).

Problem: loss = mean_b( logZ_b - gold_b ) for a linear-chain CRF with
B=512 sequences, T=512 steps, K=128 tags (START=126, STOP=127).

Algorithm (per core, data-parallel over batch, 64 sequences/core):

  Z_b = stop^T (prod_{t=T-1..0} D_t W) a0 in the exp domain, where
  W = exp(transitions - c) (c a constant per-step shift keeping magnitudes
  in bf16 range), D_t = diag(exp(feats_t)), a0 = onehot(START).

  The T=512 serial scan is the latency wall (each step is a matmul + an
  elementwise multiply, ~0.5us of fixed pipeline+semaphore latency). We
  break it with a *chunked rank-1 factorization*: products of positive
  matrices contract to rank 1 exponentially fast (top-two Lyapunov
  exponent gap ~0.1/step for these lognormal entries), so splitting T
  into C=32 chunks of L=16,

      P_c ~= (P_c 1)(1^T P_c) / (1^T P_c 1)        (interior seams)

  turns the one 512-step chain into 2C-1=63 *independent* chains of 16
  steps: a forward scan u_c = P_c 1 per chunk (chunk 0 seeded with the
  exact a0) and a backward scan v_c^T = 1^T P_c per chunk c>=1. Measured
  end-to-end seam+quantization error: rel 1.4e-4 on the loss (gate 2e-2).

  Per global step, the 63 chain states are stacked column-wise into 8
  groups of <=512 cols (one PSUM bank each): 8 matmuls (stationary
  exp(T^T-c) fwd / exp(T-c) bwd) + 8 evacuate-multiply ops split across
  the engines that can read PSUM: 4 groups go DVE tensor_tensor
  (psum * expF -> bf16), 4 groups go ACT-copy (psum -> bf16) + Pool-mult
  (bf16 * expF -> bf16), keeping all three elementwise engines busy.

  exp(feats) is precomputed on host, quantized fp8e4m3 (the DVE/Pool ops
  run 1x regardless because of the fp32 PSUM operand, so fp8 costs
  nothing and quarters the DMA: 4MB/core), and shipped ONCE in
  "both-ends-inward" pair order: pair p carries timesteps j=p and
  j=L-1-p of every chunk, which is exactly what fwd(step p)+bwd(step p)
  consume -- the scan is never DMA-starved.

  Gold-path score (transition table gather + emission gather) and the
  final log-dot seam assembly are O(B*T) host-side gathers/reductions
  on the already-present inputs (the device ships back u [128,2048] and
  v [128,1984] per core).
"""

import numpy as np
import ml_dtypes

import concourse.bass as bass
from concourse import bacc
import concourse.mybir as mybir
import concourse.tile as tile
from concourse.tile import add_dep_helper
from concourse.alu_op_type import AluOpType

B, T, K = 512, 512, 128
NCORES = 8
BPC = B // NCORES  # 64 sequences per core
START, STOP = K - 2, K - 1

C_SHIFT = 5.826096

C = 32           # chunks
L = T // C       # 16 scan steps
L2 = L // 2      # 8 DMA pair-blocks
PAIR_COLS = 2 * C * BPC   # cols per DMA pair-block (4096)
S_PROBE = 2      # backward seam-probe length (error ~ (lam2/lam1)^S per seam)

# chunk ranges per group: fwd covers chunks 0..31; probe waves cover 1..31
# groups 0+1 evacuate as one merged DVE tensor_tensor [128,1024] (psum x expf);
# groups 2+3 evacuate via one merged ACT copy psum->bf16, then the multiply
# splits: chunks 16..24 on Pool, 24..28 on Pool, 28..32 on DVE (balance).
FWD_GROUPS = [(0, 8), (8, 16), (16, 24), (24, 32)]
PROBE_WAVES = [(1, 9), (9, 17), (17, 25), (25, 32)]
WARMUP_MMS = 18  # dense PE burst in the DMA lead-in: HAM to K=8/8 (~3.4us)

F32 = mybir.dt.float32
BF16 = mybir.dt.bfloat16
FP8 = mybir.dt.float8e4

_NC_CACHE = {}


def _expf_base(p, slot, c0):
    """Column offset of (pair p, slot, chunk c0) in the expf ship layout
    [K, L2, 2, C, BPC] flattened to [K, L2*2*C*BPC]."""
    return ((p * 2 + slot) * C + c0) * BPC


def _fwd_slice(i, c0, c1):
    """expf cols multiplying the fwd chains (chunks c0:c1) at step i (t=c*L+i)."""
    p, slot = (i, 0) if i < L2 else (L - 1 - i, 1)
    return _expf_base(p, slot, c0), (c1 - c0) * BPC


def _bwd_slice(i, c0, c1):
    """expf cols multiplying the bwd chains at step i (t = c*L + L-1-i)."""
    j = L - 1 - i
    p, slot = (i, 1) if i < L2 else (j, 0)
    return _expf_base(p, slot, c0), (c1 - c0) * BPC


def build_kernel():
    key = "nc"
    if key in _NC_CACHE:
        return _NC_CACHE[key]
    nc = bacc.Bacc(None, target_bir_lowering=False)
    AF = mybir.ActivationFunctionType

    expf_d = nc.dram_tensor("expf", [K, T * BPC], FP8, kind="ExternalInput")
    # host-precomputed stationary weights exp(T^T - c) | exp(T - c), bf16
    wfb_d = nc.dram_tensor("wfb", [K, 2 * K], BF16, kind="ExternalInput")
    seed0_d = nc.dram_tensor("seed0", [K, BPC], BF16, kind="ExternalInput")
    u_d = nc.dram_tensor("u", [K, C * BPC], BF16, kind="ExternalOutput")
    v_d = nc.dram_tensor("v", [K, (C - 1) * BPC], BF16, kind="ExternalOutput")

    with tile.TileContext(nc) as tc:
        with (
            tc.tile_pool(name="const", bufs=1) as cpool,
            tc.tile_pool(name="big", bufs=1) as bigpool,
            tc.tile_pool(name="state", bufs=3) as spool,
            tc.tile_pool(name="raw", bufs=3) as rpool,
            tc.tile_pool(name="psum", bufs=1, space="PSUM") as psum_pool,
        ):
            # ---- constants: stationary weights (host pre-exponentiated) ----
            Wfb = cpool.tile([K, 2 * K], BF16)
            nc.sync.dma_start(out=Wfb, in_=wfb_d[:])
            Wf = Wfb[:, 0:K]        # exp(T^T - c): lhsT for fwd (out = W @ A)
            Wb = Wfb[:, K : 2 * K]  # exp(T  - c): lhsT for bwd (out = W^T @ m)

            # ---- resident expF, fp8, pair-block order ----
            # trigger order matters: each dma_start eats ~1us of trigger-queue
            # time, so pair 0 (the step-0/probe-seed data) goes absolutely
            # first; pairs 4-7 ride the scalar queue to parallelize triggers
            # ---- fwd seed: chunk 0 = onehot(START) (shipped), rest = 1.0 ----
            # (seed DMA issued BEFORE the 4MB expf stream: same queue, FIFO)
            seedF = cpool.tile([K, C * BPC], BF16)
            nc.vector.memset(seedF[:, BPC:], 1.0)
            nc.sync.dma_start(out=seedF[:, 0:BPC], in_=seed0_d[:])

            expf_s = bigpool.tile([K, T * BPC], FP8)
            for p in range(L2):
                nc.sync.dma_start(
                    out=expf_s[:, p * PAIR_COLS : (p + 1) * PAIR_COLS],
                    in_=expf_d[:, p * PAIR_COLS : (p + 1) * PAIR_COLS],
                )

            # ---- per-group psum banks (one each, reused across steps) ----
            # banks 0-3: fwd groups; adjacent pairs are merged-evac'd, so
            # allocate as two [K,1024] tiles (2 banks each) and slice.
            psum_f01 = psum_pool.tile([K, 16 * BPC], F32, name="pf01")
            psum_f23 = psum_pool.tile([K, 16 * BPC], F32, name="pf23")
            psum_f = [
                psum_f01[:, 0 : 8 * BPC],
                psum_f01[:, 8 * BPC : 16 * BPC],
                psum_f23[:, 0 : 8 * BPC],
                psum_f23[:, 8 * BPC : 16 * BPC],
            ]
            # two probe banks, ping-ponged by the 4 probe waves
            psum_p = [
                psum_pool.tile([K, 8 * BPC], F32, name=f"pp{w}") for w in range(2)
            ]
            psum_w = psum_pool.tile([K, 8 * BPC], F32, name="pwarm")

            fwd_state = [
                seedF[:, c0 * BPC : c1 * BPC] for (c0, c1) in FWD_GROUPS
            ]
            ustage = cpool.tile([K, 2 * 16 * BPC], BF16, name="ustage")

            # ---- PE warmup: dense MM burst while the expf DMA streams in ----
            for wi in range(WARMUP_MMS):
                nc.tensor.matmul(
                    psum_w[:, 0 : 4 * BPC],
                    seedF[:, BPC : BPC + K],
                    seedF[:, BPC : BPC + 4 * BPC],
                    start=True,
                    stop=True,
                )


            # probe seeds m0 = bf16(expf[j=L-1]) for all 4 waves: DVE casts in
            # the DMA lead-in (only pair 0 needed), so no wave ever stalls
            # the scan TTs behind it in the strict DVE FIFO
            probe_m0 = []
            for wv, (c0, c1) in enumerate(PROBE_WAVES):
                w = (c1 - c0) * BPC
                base, _ = _bwd_slice(0, c0, c1)
                m0 = cpool.tile([K, w], BF16, name=f"pm0_{wv}")
                nc.vector.tensor_copy(m0, expf_s[:, base : base + w])
                probe_m0.append(m0)

            def emit_probe_wave(wv):
                """Backward seam probe v~_c = 1^T(last S_PROBE steps of chunk c)
                for chunks [c0, c1); all ops on DVE + one ACT evac."""
                c0, c1 = PROBE_WAVES[wv]
                w = (c1 - c0) * BPC
                pp = psum_p[wv % 2][:, 0:w]
                m = probe_m0[wv]
                for i in range(S_PROBE):
                    nc.tensor.matmul(pp, Wb, m, start=True, stop=True)
                    if i < S_PROBE - 1:
                        base, _ = _bwd_slice(i + 1, c0, c1)
                        m = spool.tile(
                            [K, w], BF16, name=f"pm{wv}_{i}", tag=f"pb{wv % 2}"
                        )
                        nc.vector.tensor_mul(m, pp, expf_s[:, base : base + w])
                vt = cpool.tile([K, w], BF16, name=f"v{wv}")
                nc.scalar.copy(vt, pp)
                nc.sync.dma_start(
                    out=v_d[:, (c0 - 1) * BPC : (c1 - 1) * BPC], in_=vt
                )

            # ---- the scan: L global steps, 4 fwd chains + staggered probes ----
            W16 = 16 * BPC
            prev_tt23 = None
            for i in range(L):
                for g, (c0, c1) in enumerate(FWD_GROUPS):
                    nc.tensor.matmul(
                        psum_f[g], Wf, fwd_state[g], start=True, stop=True
                    )
                base01, _ = _fwd_slice(i, 0, 16)
                if i == L - 1:
                    new01 = ustage[:, 0:W16]
                else:
                    new01 = spool.tile([K, W16], BF16, name=f"A01_{i}", tag="af01")
                tt01 = nc.vector.tensor_mul(
                    new01, psum_f01, expf_s[:, base01 : base01 + W16]
                )
                fwd_state[0] = new01[:, 0 : 8 * BPC]
                fwd_state[1] = new01[:, 8 * BPC : W16]

                base23, _ = _fwd_slice(i, 16, 32)
                if i == L - 1:
                    new23 = ustage[:, W16 : 2 * W16]
                else:
                    new23 = spool.tile([K, W16], BF16, name=f"A23_{i}", tag="af23")
                tt23 = nc.vector.tensor_mul(
                    new23, psum_f23, expf_s[:, base23 : base23 + W16]
                )
                fwd_state[2] = new23[:, 0 : 8 * BPC]
                fwd_state[3] = new23[:, 8 * BPC : W16]
                # scheduler alternation edges: neither merged chain may run
                # ahead (a drifting chain ends the kernel latency-bound)
                add_dep_helper(
                    tt23.ins, tt01.ins, sync=False, reason="alternation"
                )
                if prev_tt23 is not None:
                    add_dep_helper(
                        tt01.ins, prev_tt23.ins, sync=False,
                        reason="alternation",
                    )
                prev_tt23 = tt23
                if i < 4:
                    emit_probe_wave(i)

            # ---- ship u (two DMAs on separate trigger queues) ----
            W16b = 16 * BPC
            nc.sync.dma_start(out=u_d[:, 0:W16b], in_=ustage[:, 0:W16b])
            nc.scalar.dma_start(
                out=u_d[:, W16b : 2 * W16b], in_=ustage[:, W16b : 2 * W16b]
            )

    nc.compile()
    nc.finalize()
    _NC_CACHE[key] = nc
    return nc


def prep_inputs(feats, tags, transitions):
    """Host-side marshalling: expF fp8 in pair-block order, shifted transitions."""
    tr = np.asarray(transitions, dtype=np.float32)
    wfb = np.exp(
        np.concatenate([np.ascontiguousarray(tr.T), tr], axis=1)
        - np.float32(C_SHIFT)
    ).astype(ml_dtypes.bfloat16)
    wfb = np.ascontiguousarray(wfb)

    seed0 = np.zeros((K, BPC), dtype=ml_dtypes.bfloat16)
    seed0[START, :] = 1.0
    feats_bf = np.asarray(feats, dtype=np.float32).astype(ml_dtypes.bfloat16)
    in_maps = []
    for core in range(NCORES):
        fc = feats_bf[core * BPC : (core + 1) * BPC]  # [BPC, T, K]
        expF16 = np.exp(fc.astype(np.float32)).astype(ml_dtypes.bfloat16)
        expF = expF16.astype(ml_dtypes.float8_e4m3)
        y = expF.transpose(2, 1, 0).reshape(K, C, L, BPC)  # [K, c, j, b]
        arr = np.empty((K, L2, 2, C, BPC), dtype=ml_dtypes.float8_e4m3)
        arr[:, :, 0] = y[:, :, :L2].transpose(0, 2, 1, 3)          # j = p
        arr[:, :, 1] = y[:, :, L - 1 : L2 - 1 : -1].transpose(0, 2, 1, 3)  # j = L-1-p
        in_maps.append(
            {"expf": np.ascontiguousarray(arr.reshape(K, T * BPC)),
             "wfb": wfb,
             "seed0": seed0}
        )
    return in_maps


def combine_outputs(results, feats, tags, transitions):
    """Host: seam assembly of logZ from u/v + exact gold score; fp64."""
    tr = np.asarray(transitions, dtype=np.float64)
    tags64 = np.asarray(tags).astype(np.int64)
    stopv = np.exp(tr[STOP, :] - C_SHIFT)  # [K]

    # gold score (exact, host)
    ext = np.concatenate(
        [np.full((B, 1), START, np.int64), tags64], axis=1
    )
    trans_gold = tr[ext[:, 1:], ext[:, :-1]].sum(axis=1) + tr[STOP, ext[:, -1]]
    emit_gold = np.take_along_axis(
        np.asarray(feats, dtype=np.float64), tags64[:, :, None], axis=2
    )[..., 0].sum(axis=1)
    gold = trans_gold + emit_gold

    total = 0.0
    for core in range(NCORES):
        u = results[core]["u"].astype(np.float64)   # [K, C*BPC]
        v = results[core]["v"].astype(np.float64)   # [K, (C-1)*BPC]
        u = u.reshape(K, C, BPC)
        v = v.reshape(K, C - 1, BPC)
        logZ = np.log(np.einsum("k,kb->b", stopv, u[:, C - 1]))
        for c in range(1, C):
            logZ += np.log(np.einsum("kb,kb->b", v[:, c - 1], u[:, c - 1]))
            logZ -= np.log(v[:, c - 1].sum(axis=0))
        logZ += (T + 1) * C_SHIFT
        total += float(np.sum(logZ - gold[core * BPC : (core + 1) * BPC]))
    return np.asarray(total / B, dtype=np.float32)


def kernel(feats, tags, transitions):
    from concourse.bass_utils import run_bass_kernel_spmd

    nc = build_kernel()
    in_maps = prep_inputs(feats, tags, transitions)
    res = run_bass_kernel_spmd(nc, in_maps, list(range(NCORES)))
    return combine_outputs(res.results, feats, tags, transitions)


if __name__ == "__main__":
    nc = build_kernel()
    print("kernel built and compiled OK")
